# revision 57
# baseline (speedup 1.0000x reference)
"""Multi-head self-attention (S=2048, B=2, D=1024, H=16) on 8 TRN2 NeuronCores.

Sharding: core c handles batch b = c//4 and head-quad g = c%4 (4 heads of 64).
Megatron-style: in_proj column-sliced, out_proj row-sliced; host sums the 8
partial outputs and adds out_proj bias.

Schedule (v2): attention runs as 8 (pair, qq) segments of 16 key-tiles; the
inner loop interleaves 64-row-tiled score pairs (both heads concurrently on
the PE row tiles), exp on the scalar engine, PV accumulation, and
fine-grained "filler" units (projections, V-projection, out-projection) so
no engine ever waits on a long blob.

  - PSUM: scores ring 2x[128,2,512] (4 banks) + u ring 3x[128,512]
    per-head accumulators (3 banks) + 1 filler slot (1 bank)
  - x stays resident in SBUF (no DMA re-streaming for the mt1
    projections); DMAs split across the two hardware-DGE queues
    (sync + scalar) in critical-path order; gpsimd DMA (slow software
    path) is never on the critical path
  - v-proj bias folded into the DVE psum drain (no bias matmuls)
  - softmax normalization: copy row-sums off partition 64, DVE
    reciprocal, GPSIMD partition_broadcast, DVE multiply straight out of
    PSUM (no broadcast matmul, no extra PSUM bank); each segment flushes
    in its epilogue so the chain overlaps the next segment's compute
  - output stored bf16; final out-proj tiles split across both DMA queues
"""

import math
from contextlib import ExitStack

import numpy as np

S = 2048
B = 2
D = 1024
H = 16
DK = 64
HC = 4          # heads per core
M = HC * DK     # 256 head-dim columns per core
N_CORES = 8
KT = S // 128   # 16 key tiles
QQ = 4          # 512-wide query chunks

MM_DT = "bfloat16"

_compiled = None


def _build_program():
    import concourse.tile as tile
    from concourse import mybir, bacc

    f32 = mybir.dt.float32
    mdt = getattr(mybir.dt, MM_DT)
    EXP = mybir.ActivationFunctionType.Exp
    ADD = mybir.AluOpType.add
    MULT = mybir.AluOpType.mult

    nc = bacc.Bacc("TRN2", target_bir_lowering=False, debug=False)

    xqT = nc.dram_tensor("xqT", [D, S], mdt, kind="ExternalInput").ap()
    xkT = nc.dram_tensor("xkT", [D, S], mdt, kind="ExternalInput").ap()
    xvT = nc.dram_tensor("xvT", [D, S], mdt, kind="ExternalInput").ap()
    wqT = nc.dram_tensor("wqT", [D, M], mdt, kind="ExternalInput").ap()
    wkT = nc.dram_tensor("wkT", [D, M], mdt, kind="ExternalInput").ap()
    wvT = nc.dram_tensor("wvT", [D, M], mdt, kind="ExternalInput").ap()
    bqk = nc.dram_tensor("bqk", [128, 4], f32, kind="ExternalInput").ap()
    bv = nc.dram_tensor("bv", [1, 2 * M], f32, kind="ExternalInput").ap()
    woT = nc.dram_tensor("woT", [M, D], mdt, kind="ExternalInput").ap()
    out = nc.dram_tensor("out", [S, D], mdt, kind="ExternalOutput").ap()

    with tile.TileContext(nc) as tc, ExitStack() as ctx:
        const_pool = ctx.enter_context(tc.tile_pool(name="const", bufs=1))
        e_pool = ctx.enter_context(tc.tile_pool(name="e", bufs=10))
        ot_pool = ctx.enter_context(tc.tile_pool(name="ot", bufs=4))
        r_pool = ctx.enter_context(tc.tile_pool(name="r", bufs=3))
        ps_sc = ctx.enter_context(tc.tile_pool(name="ps_sc", bufs=2, space="PSUM"))
        ps_u = ctx.enter_context(tc.tile_pool(name="ps_u", bufs=3, space="PSUM"))
        ps_f = ctx.enter_context(tc.tile_pool(name="ps_f", bufs=1, space="PSUM"))

        # ---- persistent SBUF tensors ----
        wq_sb = const_pool.tile([128, 8, M], mdt)
        wk_sb = const_pool.tile([128, 8, M], mdt)
        wv_sb = const_pool.tile([128, 8, M], mdt)
        wo_sb = const_pool.tile([128, 2, D], mdt)
        bqk_sb = const_pool.tile([128, 4], f32)
        bv_row = const_pool.tile([1, 2 * M], f32)
        bvb = const_pool.tile([128, 2 * M], f32)

        qpT = const_pool.tile([128, 2, S], mdt)   # [p, mt, s]
        kpT = const_pool.tile([128, 2, S], mdt)
        vp = const_pool.tile([128, KT, HC * 65], mdt)  # aug: 65-wide per head
        attnT = const_pool.tile([128, 2, S], mdt)

        # ones columns of the augmented V (head h's ones at column h*65+64)
        nc.vector.memset(
            vp[:, :, :].rearrange("p kt (h c) -> p kt h c", c=65)[:, :, :, 64:65], 1.0
        )

        # x stays resident in SBUF for the whole kernel
        xk_sb = const_pool.tile([128, 2, 8, 1024], mdt)
        xq_sb = const_pool.tile([128, 2, 8, 1024], mdt)
        xv_sb = const_pool.tile([128, 2, 8, 1024], mdt)
        chunks_k = [xk_sb[:, i // 8, i % 8, :] for i in range(16)]
        chunks_q = [xq_sb[:, i // 8, i % 8, :] for i in range(16)]
        chunks_v = [xv_sb[:, i // 8, i % 8, :] for i in range(16)]

        # ---- DMA emission: few big transfers, split across the two
        # hardware-DGE queues (sync + scalar); gpsimd DMA is a slow
        # software path — keep it off the critical path entirely.
        def load_half(x_sb, x_dr, half, eng, pieces=1):
            fs = half * 1024
            kq = 8 // pieces
            for pc in range(pieces):
                eng.dma_start(
                    out=x_sb[:, half, pc * kq:(pc + 1) * kq, :],
                    in_=x_dr[pc * kq * 128:(pc + 1) * kq * 128, fs:fs + 1024]
                    .rearrange("(kc p) s -> p kc s", p=128),
                )

        # preload the ACT exp table during the startup dead zone so the
        # first real exp doesn't pay the lazy ACT_TABLE_LOAD on the
        # critical path (memset on gpsimd so no engine on the critical
        # path is touched)
        warm_in = const_pool.tile([1, 8], f32)
        warm_out = const_pool.tile([1, 8], f32)
        nc.gpsimd.memset(warm_in[:, :], 0.0)
        nc.scalar.activation(out=warm_out[:, :], in_=warm_in[:, :], func=EXP)

        # scalar: wk, wq, xq-h0 chunks, biases (ACT idle until first exp);
        # sync: xk-h0 chunks, wv, xv-h0, then the later halves + wo.
        # Emission order doubles as the global DMA-semaphore-ring order:
        # keep late-consumed loads (biases, wo) behind the critical ones so
        # ring reuse never stalls an early stream on a slow consumer.
        nc.scalar.dma_start(
            out=wk_sb[:, :, :], in_=wkT.rearrange("(kc p) m -> p kc m", p=128)
        )
        nc.scalar.dma_start(
            out=wq_sb[:, :, :], in_=wqT.rearrange("(kc p) m -> p kc m", p=128)
        )
        # critical halves in 512KB pieces; both queues race the 5MB the
        # first scores need (xk-h0 + xq-h0) before anything non-critical
        def x_piece(x_sb, x_dr, pc, eng):
            kq = 2
            eng.dma_start(
                out=x_sb[:, 0, pc * kq:(pc + 1) * kq, :],
                in_=x_dr[pc * kq * 128:(pc + 1) * kq * 128, 0:1024]
                .rearrange("(kc p) s -> p kc s", p=128),
            )

        for pc in range(4):
            x_piece(xk_sb, xkT, pc, nc.sync)
            x_piece(xq_sb, xqT, pc, nc.scalar)
        nc.scalar.dma_start(out=bqk_sb[:, :], in_=bqk[:, :])
        nc.scalar.dma_start(out=bv_row[:, :], in_=bv[:, :])
        nc.gpsimd.partition_broadcast(bvb[:, :], bv_row[0:1, :], channels=128)
        nc.sync.dma_start(
            out=wv_sb[:, :, :], in_=wvT.rearrange("(kc p) m -> p kc m", p=128)
        )
        load_half(xv_sb, xvT, 0, nc.sync, pieces=2)
        load_half(xk_sb, xkT, 1, nc.sync, pieces=2)
        load_half(xv_sb, xvT, 1, nc.sync, pieces=2)
        load_half(xq_sb, xqT, 1, nc.sync, pieces=2)
        nc.sync.dma_start(
            out=wo_sb[:, :, :], in_=woT.rearrange("(kc p) j -> p kc j", p=128)
        )

        # ---- filler units (generators; each yield ~= 2 matmuls or a drain) ----
        def gen_proj(pT, w_sb, b_off, mt, half, nch, chunks, pool=None, tag=None):
            fs, ns = half * 1024, nch * 512
            ps = (pool or ps_f).tile(
                [128, 512], f32, tag=tag or "f", name=f"pp{mt}{half}{nch}"
            )
            for kc in range(8):
                nc.tensor.matmul(
                    ps[:, :],
                    w_sb[:, kc, mt * 128:(mt + 1) * 128],
                    chunks[half * 8 + kc][:, ns:ns + 512],
                    start=(kc == 0),
                    stop=(kc == 7),
                )
                if kc % 2 == 1:
                    yield
            nc.vector.tensor_scalar_add(
                out=pT[:, mt, fs + ns:fs + ns + 512],
                in0=ps[:, :],
                scalar1=bqk_sb[:, b_off + mt:b_off + mt + 1],
            )
            yield

        def gen_vp2(kt0):
            # two kt of V-projection per unit: one psum slot, one drain
            ps = ps_f.tile([128, 512], f32, tag="f", name=f"pv{kt0}")
            for j in range(2):
                kt = kt0 + j
                half, st = divmod(kt, 8)
                for kc in range(8):
                    nc.tensor.matmul(
                        ps[:, j * M:j * M + M],
                        chunks_v[half * 8 + kc][:, st * 128:(st + 1) * 128],
                        wv_sb[:, kc, :],
                        start=(kc == 0),
                        stop=(kc == 7),
                    )
                    if kc % 2 == 1:
                        yield
            # bias added during the psum drain (broadcast tile, DVE)
            nc.vector.tensor_tensor(
                out=vp[:, kt0:kt0 + 2, :].rearrange(
                    "p kt (h c) -> p kt h c", c=65
                )[:, :, :, 0:64],
                in0=ps[:, :].rearrange("p (kt h c) -> p kt h c", kt=2, c=64),
                in1=bvb[:, :].rearrange("p (kt h c) -> p kt h c", kt=2, c=64),
                op=ADD,
            )
            yield

        def gen_outproj(sg, pool=None, tag=None, eng=None):
            ot = ot_pool.tile([128, D], mdt, tag="ot", name=f"ot{sg}")
            for nch in range(2):
                ns = nch * 512
                po = (pool or ps_f).tile(
                    [128, 512], f32, tag=tag or "f", name=f"po{sg}{nch}"
                )
                for kc in range(2):
                    nc.tensor.matmul(
                        po[:, :],
                        attnT[:, kc, sg * 128:(sg + 1) * 128],
                        wo_sb[:, kc, ns:ns + 512],
                        start=(kc == 0),
                        stop=(kc == 1),
                    )
                with nc.allow_low_precision(reason="bf16 output"):
                    nc.vector.tensor_copy(out=ot[:, ns:ns + 512], in_=po[:, :])
                yield
            (eng or nc.sync).dma_start(
                out=out[sg * 128:(sg + 1) * 128, :], in_=ot[:, :]
            )
            yield

        def run_full(gen):
            for _ in gen:
                pass

        # ---- flush: normalize a head's accumulated PV into attnT ----
        def emit_flush(pair, qq, hh, u):
            qs = qq * 512
            rs = r_pool.tile([1, 512], f32, tag="rs")
            nc.vector.tensor_copy(out=rs[:, :], in_=u[64:65, :])
            rbi = r_pool.tile([1, 512], f32, tag="rbi")
            with nc.allow_low_precision(reason="softmax denom"):
                nc.vector.reciprocal_approx_fast(out=rbi[:, :], in_=rs[0:1, :])
            rbb = r_pool.tile([64, 512], f32, tag="rbb")
            nc.gpsimd.partition_broadcast(rbb[0:64, :], rbi[0:1, :], channels=64)
            with nc.allow_low_precision(reason="softmax normalize"):
                nc.vector.tensor_tensor(
                    out=attnT[hh * 64:hh * 64 + 64, pair, qs:qs + 512],
                    in0=u[0:64, :],
                    in1=rbb[0:64, :],
                    op=MULT,
                )

        # ---- preamble projections (needed before the first scores) ----
        # kpT-nch0 and qpT-c0 interleaved chunk-by-chunk (their x streams
        # land on different DMA queues in parallel); qpT uses a scores
        # psum slot so the two chains overlap
        g1 = gen_proj(kpT, wk_sb, 2, 0, 0, 0, chunks_k)
        g2 = gen_proj(qpT, wq_sb, 0, 0, 0, 0, chunks_q, pool=ps_sc, tag="sc")
        done1 = done2 = False
        while not (done1 and done2):
            if not done1:
                done1 = next(g1, StopIteration) is StopIteration
            if not done2:
                done2 = next(g2, StopIteration) is StopIteration

        # ---- attention master loop ----
        # work deque of filler generators, consumed in order
        work = []
        work_budget_steps = 2

        def advance(n):
            while n > 0 and work:
                try:
                    next(work[0])
                    n -= 1
                except StopIteration:
                    work.pop(0)

        def S_pair(pair, qq, kt, sc):
            qs = qq * 512
            ks = kt * 128
            for hh in range(2):
                po = hh * 64
                nc.tensor.matmul(
                    sc[:, hh, :],
                    kpT[po:po + 64, pair, ks:ks + 128],
                    qpT[po:po + 64, pair, qs:qs + 512],
                    start=True,
                    stop=True,
                )

        def PV(pair, kt, hh, u, et):
            h = 2 * pair + hh
            nc.tensor.matmul(
                u[0:65, :],
                vp[:, kt, h * 65:(h + 1) * 65],
                et[:, hh, :],
                start=(kt == 0),
                stop=(kt == KT - 1),
            )

        SEGS = [(p, q) for p in range(2) for q in range(4)]
        pending_flush = None   # (pair, qq, [u_h0, u_h1])

        for si, (pair, qq) in enumerate(SEGS):
            # add this segment's filler units to the deque
            if si == 1:
                work.append(gen_proj(qpT, wq_sb, 0, 0, 1, 0, chunks_q))  # c2
                work.append(gen_proj(kpT, wk_sb, 2, 1, 0, 0, chunks_k))
                work.append(gen_proj(kpT, wk_sb, 2, 1, 0, 1, chunks_k))
            elif si == 2:
                work.append(gen_proj(qpT, wq_sb, 0, 0, 1, 1, chunks_q))  # c3
                work.append(gen_proj(kpT, wk_sb, 2, 1, 1, 0, chunks_k))
                work.append(gen_proj(kpT, wk_sb, 2, 1, 1, 1, chunks_k))
            elif si == 3:
                work.append(gen_proj(qpT, wq_sb, 0, 1, 0, 0, chunks_q))
                work.append(gen_proj(qpT, wq_sb, 0, 1, 0, 1, chunks_q))
            elif si == 4:
                work.append(gen_proj(qpT, wq_sb, 0, 1, 1, 0, chunks_q))
            elif si == 5:
                work.append(gen_proj(qpT, wq_sb, 0, 1, 1, 1, chunks_q))
            if pair == 1 and qq >= 1:
                for sg in range((qq - 1) * 4, qq * 4):
                    work.append(gen_outproj(sg))

            u_tiles = [None, None]
            et_tiles = {}
            for g in range(KT // 2):
                kts = (2 * g, 2 * g + 1)
                sc = ps_sc.tile([128, 2, 512], f32, tag="sc", name=f"sc{si}_{g}")
                S_pair(pair, qq, kts[0], sc)
                et0 = e_pool.tile([128, 2, 512], mdt, tag="et")
                nc.scalar.activation(out=et0[:, :, :], in_=sc[:, :, :], func=EXP)
                et_tiles[kts[0]] = et0

                sc2 = ps_sc.tile([128, 2, 512], f32, tag="sc", name=f"sc{si}_{g}b")
                S_pair(pair, qq, kts[1], sc2)
                et1 = e_pool.tile([128, 2, 512], mdt, tag="et")
                nc.scalar.activation(out=et1[:, :, :], in_=sc2[:, :, :], func=EXP)
                et_tiles[kts[1]] = et1

                if si == 0:
                    # V-projection just-in-time: vp(kt) ready one group
                    # before its first PV consumer
                    run_full(gen_vp2(kts[0]))

                # PVs of the previous group
                if g > 0:
                    for kt in (2 * g - 2, 2 * g - 1):
                        if u_tiles[0] is None:
                            u_tiles[0] = ps_u.tile(
                                [128, 512], f32, tag="u", name=f"u{si}_0"
                            )
                            u_tiles[1] = ps_u.tile(
                                [128, 512], f32, tag="u", name=f"u{si}_1"
                            )
                        PV(pair, kt, 0, u_tiles[0], et_tiles[kt])
                        PV(pair, kt, 1, u_tiles[1], et_tiles[kt])
                        del et_tiles[kt]

                if si == 0:
                    if g == 0:
                        # kpT mt0 nch1 needed by S(kt4) at g2
                        run_full(gen_proj(kpT, wk_sb, 2, 0, 0, 1, chunks_k))
                    elif g in (2, 3):
                        # kpT mt0 half1 needed by S(kt8) at g4
                        run_full(
                            gen_proj(kpT, wk_sb, 2, 0, 1, g - 2, chunks_k)
                        )
                    elif g == 6:
                        # qpT mt0 c1 needed by segment 1
                        run_full(
                            gen_proj(qpT, wq_sb, 0, 0, 0, 1, chunks_q)
                        )
                else:
                    # drain the last segment's fillers early so the tail
                    # after the final flush is just the last out-proj
                    advance(6 if si == 7 else (work_budget_steps if pair == 0 else 3))

            # segment epilogue: last PVs, flushing each head as its
            # accumulation completes so the DVE chain overlaps PE work
            PV(pair, KT - 2, 0, u_tiles[0], et_tiles[KT - 2])
            PV(pair, KT - 1, 0, u_tiles[0], et_tiles[KT - 1])
            emit_flush(pair, qq, 0, u_tiles[0])
            PV(pair, KT - 2, 1, u_tiles[1], et_tiles[KT - 2])
            PV(pair, KT - 1, 1, u_tiles[1], et_tiles[KT - 1])
            emit_flush(pair, qq, 1, u_tiles[1])
            del et_tiles[KT - 2], et_tiles[KT - 1]

        # tail: drain leftovers, then the last out-projection chunk.
        # kc=0 (pair-0 rows of attnT) doesn't depend on the final flush —
        # pre-accumulate it across every spare PSUM slot while the flush's
        # DVE chain runs, then finish with kc=1 and drain.
        while work:
            advance(1000)
        po12 = ps_sc.tile([128, 2, 512], f32, tag="sc", name="po12")
        po13 = ps_sc.tile([128, 2, 512], f32, tag="sc", name="po13")
        po14 = [ps_u.tile([128, 512], f32, tag="u", name=f"po14{n}")
                for n in range(2)]
        po15 = [ps_u.tile([128, 512], f32, tag="u", name="po150"),
                ps_f.tile([128, 512], f32, tag="f", name="po151")]
        halves = (
            [(po12[:, n, :], 12, n * 512) for n in range(2)]
            + [(po13[:, n, :], 13, n * 512) for n in range(2)]
            + [(po14[n][:, :], 14, n * 512) for n in range(2)]
            + [(po15[n][:, :], 15, n * 512) for n in range(2)]
        )
        for kc in range(2):
            for ps, sg, ns in halves:
                nc.tensor.matmul(
                    ps,
                    attnT[:, kc, sg * 128:(sg + 1) * 128],
                    wo_sb[:, kc, ns:ns + 512],
                    start=(kc == 0),
                    stop=(kc == 1),
                )
        ots = {sg: ot_pool.tile([128, D], mdt, tag="ot", name=f"ot{sg}")
               for sg in (12, 13, 14, 15)}
        for ps, sg, ns in halves:
            with nc.allow_low_precision(reason="bf16 output"):
                nc.vector.tensor_copy(out=ots[sg][:, ns:ns + 512], in_=ps)
        for idx, sg in enumerate((12, 13, 14, 15)):
            eng = nc.sync if idx % 2 == 0 else nc.scalar
            eng.dma_start(out=out[sg * 128:(sg + 1) * 128, :], in_=ots[sg][:, :])

    nc.compile()
    return nc


def _get_compiled():
    global _compiled
    if _compiled is None:
        _compiled = _build_program()
    return _compiled


def _make_in_maps(q, k, v, in_proj_w, in_proj_b, out_proj_w):
    import ml_dtypes

    mdt_np = np.dtype(ml_dtypes.bfloat16) if MM_DT == "bfloat16" else np.float32

    def cvt(a):
        return np.ascontiguousarray(a).astype(mdt_np)

    xT = {}
    for b in range(B):
        xT[b] = (
            cvt(q[:, b, :].T),
            cvt(k[:, b, :].T),
            cvt(v[:, b, :].T),
        )
    scale = 1.0 / math.sqrt(DK)
    in_maps = []
    for c in range(N_CORES):
        b, g = divmod(c, HC)
        cols = slice(g * M, (g + 1) * M)
        in_maps.append({
            "xqT": xT[b][0],
            "xkT": xT[b][1],
            "xvT": xT[b][2],
            "wqT": cvt((in_proj_w[0 * D:1 * D][cols] * scale).T),
            "wkT": cvt(in_proj_w[1 * D:2 * D][cols].T),
            "wvT": cvt(in_proj_w[2 * D:3 * D][cols].T),
            "bqk": np.ascontiguousarray(np.concatenate([
                (in_proj_b[0 * D:1 * D][cols] * scale).reshape(2, 128).T,
                in_proj_b[1 * D:2 * D][cols].reshape(2, 128).T,
            ], axis=1)).astype(np.float32),
            "bv": np.ascontiguousarray(
                np.tile(in_proj_b[2 * D:3 * D][cols], 2).reshape(1, 2 * M)
            ).astype(np.float32),
            "woT": cvt(out_proj_w[:, g * M:(g + 1) * M].T),
        })
    return in_maps


def kernel(q, k, v, in_proj_w, in_proj_b, out_proj_w, out_proj_b):
    from concourse.bass_utils import run_bass_kernel_spmd

    q = np.asarray(q, dtype=np.float32)
    k = np.asarray(k, dtype=np.float32)
    v = np.asarray(v, dtype=np.float32)
    in_proj_w = np.asarray(in_proj_w, dtype=np.float32)
    in_proj_b = np.asarray(in_proj_b, dtype=np.float32)
    out_proj_w = np.asarray(out_proj_w, dtype=np.float32)
    out_proj_b = np.asarray(out_proj_b, dtype=np.float32)

    nc = _get_compiled()
    in_maps = _make_in_maps(q, k, v, in_proj_w, in_proj_b, out_proj_w)

    res = run_bass_kernel_spmd(nc, in_maps, core_ids=list(range(N_CORES)))

    out = np.broadcast_to(out_proj_b.astype(np.float32), (S, B, D)).copy()
    for c in range(N_CORES):
        out[:, c // HC, :] += res.results[c]["out"].astype(np.float32)
    return out
